# revision 3
# baseline (speedup 1.0000x reference)
"""Trainium2 Bass kernel for MemoryEfficientCrossAttention (fused bf16).

Problem (hardcoded): B=2, Q=2048, K=4096, HIDDEN=1024, HEADS=16, HEAD_DIM=64.
  out = softmax((x_q W_q)(x_k W_k)^T / sqrt(64)) (x_v W_v) W_o

Sharding over 8 NeuronCores: core = g*4 + r
  g in {0,1}: head-group (8 heads -> 512 cols of W_q/W_k/W_v)
  r in {0..3}: 1024-row block of the flattened (B*Q, H) query (batch r//2)

Host pre-transposes activations (hidden-major) and converts everything to
bf16, so the device never transposes.  The K/V projections are FUSED into
the attention sweep: attention runs kb-segment-major (8 segments of 4
k-blocks); within a segment every (strip, q-block) pair computes scores ->
exp -> PV for those k-blocks, accumulating context in PSUM for the segment
and folding it into an SBUF f32 accumulator; meanwhile the next segment's
K/V projection matmuls are interleaved into the PE stream, so the
Activation engine (the exp bottleneck) starts ~100us earlier than a
phase-separated schedule.  Softmax denominators come from a ones-column
appended to V.  The (g=0,g=1) pair AllGathers the normalized context in
bf16 and each core computes its own 512-column half of the W_o product
(the host stitches the halves).
"""

import sys
import time

import numpy as np

sys.path.insert(0, "/opt/trn_rl_repo")

import concourse.mybir as mybir  # noqa: E402
import concourse.tile as tile  # noqa: E402
from concourse import bacc  # noqa: E402

try:
    import ml_dtypes  # noqa: E402
    BF16_NP = ml_dtypes.bfloat16
except ImportError:
    import jax.numpy as jnp  # noqa: E402
    BF16_NP = jnp.bfloat16

F32 = mybir.dt.float32
BF16 = mybir.dt.bfloat16

HID = 1024
HEADS = 16
HD = 64
B = 2
Q = 2048
KL = 4096
NCORE = 8
GC = 512          # head-group cols per core (8 heads)
QR = 1024         # query rows per core
OC = 512          # out-proj cols per core (g-half of HID)
SCALE = HD ** -0.5

_CACHED_NC = None


def _build(repeat=1):
    nc = bacc.Bacc("TRN2", target_bir_lowering=False, debug=False,
                   num_devices=NCORE)

    qT = nc.dram_tensor("qT", [HID, QR], BF16, kind="ExternalInput")
    kT = nc.dram_tensor("kT", [HID, KL], BF16, kind="ExternalInput")
    vT = nc.dram_tensor("vT", [HID, KL], BF16, kind="ExternalInput")
    wq_s = nc.dram_tensor("wq_s", [HID, GC], BF16, kind="ExternalInput")
    wk_s = nc.dram_tensor("wk_s", [HID, GC], BF16, kind="ExternalInput")
    wv_s = nc.dram_tensor("wv_s", [HID, GC], BF16, kind="ExternalInput")
    wo_s = nc.dram_tensor("wo_s", [HID, OC], BF16, kind="ExternalInput")
    outT_h = nc.dram_tensor("outT_h", [OC, QR], F32, kind="ExternalOutput")

    NKB = KL // 128           # 32 k-blocks
    NCH = HID // 128          # 8 hidden chunks
    NSEG = 8                  # kb segments (4 k-blocks each)
    SKB = NKB // NSEG         # 4 k-blocks per segment

    from contextlib import ExitStack

    with tile.TileContext(nc, pool_alloc_mode="queue") as tc:
        with tc.tile_pool(name="dram", bufs=1, space="DRAM") as dram:
            _pst = ExitStack()
            pp = _pst.enter_context(tc.tile_pool(name="persist", bufs=1))
            ctx_own = dram.tile([GC, QR], BF16)
            ctx_gath = dram.tile([2, GC, QR], BF16)
            qTh = pp.tile([128, 4, QR], BF16)     # [pair cols, strip, q rows]
            kTh = pp.tile([128, 4, KL], BF16)
            v_aug = pp.tile([128, NKB, 8, HD + 1], BF16)
            # f32 context accumulator rows: 0..63 ctx, 64 denominator
            ctxacc = pp.tile([HD + 1, 4, 2, 2, 512], F32, name="ctxacc")
            wo_sb = pp.tile([128, NCH, OC], BF16, name="wo_sb")
            wk_sb = pp.tile([128, NCH, GC], BF16, name="wk_sb")
            wv_sb = pp.tile([128, NCH, GC], BF16, name="wv_sb")
            wq_sb = pp.tile([128, NCH, GC], BF16, name="wq_sb")

            ones = pp.tile([128, NKB * 8], BF16, name="ones")
            nc.vector.memset(ones[:], 1.0)
            nc.vector.tensor_copy(
                v_aug[:, :, :, HD],
                ones[:].rearrange("p (a b) -> p a b", a=NKB))

            nc.sync.dma_start(
                wk_sb[:], wk_s[:].rearrange("(hc p) c -> p hc c", p=128))
            nc.sync.dma_start(
                wv_sb[:], wv_s[:].rearrange("(hc p) c -> p hc c", p=128))
            nc.gpsimd.dma_start(
                wo_sb[:], wo_s[:].rearrange("(hc p) c -> p hc c", p=128))
            nc.sync.dma_start(
                wq_sb[:], wq_s[:].rearrange("(hc p) c -> p hc c", p=128))

            with (
                tc.tile_pool(name="ctxp", bufs=1) as ctxp,
                tc.tile_pool(name="apool", bufs=3) as apool,
                tc.tile_pool(name="misc", bufs=2) as misc,
                tc.tile_pool(name="xstage", bufs=1) as xs,
                tc.tile_pool(name="ctxgp", bufs=1) as ctxgp,
                tc.tile_pool(name="osb", bufs=2) as osb,
            ):
              for _rep in range(repeat):
                ctxall = ctxp.tile([128, 4, QR], BF16, tag="ctxall",
                                   name=f"ctxall_{_rep}")
                xs_stack = ExitStack()
                pst = xs_stack.enter_context(
                    tc.tile_pool(name="pst", bufs=1, space="PSUM"))
                pctx = xs_stack.enter_context(
                    tc.tile_pool(name="pctx", bufs=1, space="PSUM"))
                pj = xs_stack.enter_context(
                    tc.tile_pool(name="pj", bufs=2, space="PSUM"))

                def load_xT(src, c0, tag):
                    xt = xs.tile([128, NCH, 512], BF16, tag=tag)
                    nc.sync.dma_start(
                        xt[:],
                        src[:, c0:c0 + 512].rearrange(
                            "(hc p) q -> p hc q", p=128))
                    return xt

                # ---- prologue: q projection + K/V chunk 0 ----
                for c in range(QR // 512):
                    xt = load_xT(qT, c * 512, "xk")
                    for s in range(4):
                        ps = pj.tile([128, 512], F32, tag="pj")
                        for hc in range(NCH):
                            nc.tensor.matmul(
                                ps[:],
                                wq_sb[:, hc, s * 128:(s + 1) * 128],
                                xt[:, hc, :],
                                start=(hc == 0), stop=(hc == NCH - 1))
                        nc.vector.tensor_copy(
                            qTh[:, s, c * 512:(c + 1) * 512], ps[:])

                def kv_units(c):
                    """Emission units for K/V projection of chunk c
                    (k rows c*512..c*512+512)."""
                    xtk = load_xT(kT, c * 512, "xk")
                    xtv = load_xT(vT, c * 512, "xv")
                    units = []

                    def k_unit(s):
                        def emit():
                            ps = pj.tile([128, 512], F32, tag="pj")
                            for hc in range(NCH):
                                nc.tensor.matmul(
                                    ps[:],
                                    wk_sb[:, hc, s * 128:(s + 1) * 128],
                                    xtk[:, hc, :],
                                    start=(hc == 0), stop=(hc == NCH - 1))
                            nc.vector.tensor_copy(
                                kTh[:, s, c * 512:(c + 1) * 512], ps[:])
                        return emit

                    def v_unit(t):
                        def emit():
                            ps = pj.tile([128, GC], F32, tag="pj")
                            for hc in range(NCH):
                                nc.tensor.matmul(
                                    ps[:],
                                    xtv[:, hc, t * 128:(t + 1) * 128],
                                    wv_sb[:, hc, :],
                                    start=(hc == 0), stop=(hc == NCH - 1))
                            nc.vector.tensor_copy(
                                v_aug[:, c * 4 + t, :, 0:HD],
                                ps[:].rearrange("p (h d) -> p h d", h=8))
                        return emit

                    for s in range(4):
                        units.append(k_unit(s))
                    for t in range(4):
                        units.append(v_unit(t))
                    return units

                for u in kv_units(0):
                    u()

                # ---- fused attention sweep ----
                for seg in range(NSEG):
                    # next segment's K/V projection, interleaved one unit
                    # per (s, qb) slot (8 units per chunk, 8 slots)
                    units = kv_units(seg + 1) if seg + 1 < NSEG else []
                    ui = 0
                    for it in range(8):
                        s, qb = it // 2, it % 2
                        qsl = slice(qb * 512, (qb + 1) * 512)
                        ctxs = [pctx.tile([HD + 1, 512], F32, tag=f"ctx{i}",
                                          name=f"c{i}_{s}_{qb}_{seg}_{_rep}")
                                for i in range(2)]
                        for grp in range(2):
                            kbs = [seg * SKB + grp * 2 + j for j in range(2)]
                            sts = [pst.tile([128, 1024], F32, tag=f"st{i}",
                                            name=f"s{i}_{it}_{seg}_{grp}_{_rep}")
                                   for i in range(2)]
                            for j, kb in enumerate(kbs):
                                ksl = slice(kb * 128, (kb + 1) * 128)
                                jsl = slice(j * 512, (j + 1) * 512)
                                for i in range(2):
                                    psl = slice(i * 64, (i + 1) * 64)
                                    nc.tensor.matmul(
                                        sts[i][:, jsl],
                                        kTh[psl, s, ksl],
                                        qTh[psl, s, qsl],
                                        start=True, stop=True)
                            As = []
                            for i in range(2):
                                a = apool.tile([128, 1024], BF16,
                                               tag=f"A{i}",
                                               name=f"a{i}_{it}_{seg}_{grp}_{_rep}")
                                nc.scalar.activation(
                                    a[:], sts[i][:],
                                    mybir.ActivationFunctionType.Exp,
                                    scale=SCALE)
                                As.append(a)
                            # backfill PE while exp runs
                            if ui < len(units):
                                units[ui]()
                                ui += 1
                            for j, kb in enumerate(kbs):
                                jsl = slice(j * 512, (j + 1) * 512)
                                for i in range(2):
                                    nc.tensor.matmul(
                                        ctxs[i][:],
                                        v_aug[:, kb, 2 * s + i, :],
                                        As[i][:, jsl],
                                        start=(kb == kbs[0] and grp == 0),
                                        stop=(kb == kbs[1] and grp == 1))
                        for i in range(2):
                            if seg == 0:
                                nc.vector.tensor_copy(
                                    ctxacc[:, s, qb, i, :], ctxs[i][:])
                            else:
                                nc.vector.tensor_add(
                                    ctxacc[:, s, qb, i, :],
                                    ctxacc[:, s, qb, i, :], ctxs[i][:])
                        if seg == NSEG - 1:
                            # normalize this (s, qb) as soon as its last
                            # accumulate lands; DMA each finished strip out
                            # so only the collective remains in the tail
                            for i in range(2):
                                rinv = misc.tile([1, 512], F32, tag="rinv")
                                nc.vector.reciprocal(
                                    rinv[:], ctxacc[HD:HD + 1, s, qb, i, :])
                                rb = misc.tile([HD, 512], F32, tag="rb")
                                nc.gpsimd.partition_broadcast(rb[:], rinv[:])
                                nc.vector.tensor_mul(
                                    ctxall[i * 64:(i + 1) * 64, s, qsl],
                                    ctxacc[0:HD, s, qb, i, :], rb[:])
                            if qb == 1:
                                nc.sync.dma_start(
                                    ctx_own[s * 128:(s + 1) * 128, :],
                                    ctxall[:, s, :])

                xs_stack.close()

                # ---- exchange + output projection ----
                nc.gpsimd.collective_compute(
                    "AllGather", mybir.AluOpType.bypass,
                    ins=[ctx_own[:]],
                    outs=[ctx_gath[:]],
                    replica_groups=[[0, 4], [1, 5], [2, 6], [3, 7]])

                with tc.tile_pool(name="pwo", bufs=4,
                                  space="PSUM") as pwo:
                    ctxg = ctxgp.tile([128, NCH, QR], BF16, tag="ctxg",
                                      name=f"ctxg_{_rep}")
                    for gp in range(2):
                        nc.sync.dma_start(
                            ctxg[:, gp * 4:(gp + 1) * 4, :],
                            ctx_gath[gp].rearrange("(s p) q -> p s q", p=128))
                    for oc in range(OC // 128):
                        for half in range(2):
                            ps = pwo.tile([128, 512], F32, tag="po")
                            hsl = slice(half * 512, (half + 1) * 512)
                            for hc in range(NCH):
                                nc.tensor.matmul(
                                    ps[:],
                                    wo_sb[:, hc, oc * 128:(oc + 1) * 128],
                                    ctxg[:, hc, hsl],
                                    start=(hc == 0), stop=(hc == NCH - 1))
                            ot = osb.tile([128, 512], F32, tag="ot")
                            nc.vector.tensor_copy(ot[:], ps[:])
                            nc.sync.dma_start(
                                outT_h[oc * 128:(oc + 1) * 128, hsl], ot[:])

            _pst.close()

    nc.compile()
    return nc
